# revision 13
# baseline (speedup 1.0000x reference)
"""GCN encoder (2-layer) on 8 Trainium2 NeuronCores.

Strategy: the GCN layer out = A_hat @ (x @ W + b) with
A_hat = D^-1/2 (A + I) D^-1/2 is cast as dense matmuls on the tensor
engine.  The count matrix C = A + I (20000x20000, 0.16% dense) is
materialized host-side in fp8 (small integer counts -> exact) from the
edge list (index-only preprocessing), node-partitioned column-blocks
across the 8 cores.  The D^-1/2 factors fold into cheap per-row /
per-column scalings at PSUM eviction (relu commutes with the positive
scaling), and biases fold in as rank-1 updates b (x) s with
s[c] = sum_r A_hat[c, r] host-computed.  Per core:

  MM1   z1' = dinv * (x @ W1)             (replicated, node-major)
  AGG1  hT  = relu(dinv * contract_k(z1', C_blk) + b1 (x) s)
  MM3   z2' = dinv * (h_blk @ W2)         (node-major local block)
  AllGather z2' blocks across cores
  AGG2  outT = dinv * contract_k(z2', C_blk) + b2 (x) s

C_blk is the kxn operand of both aggregations with M_TILES == 1, so it
streams from HBM exactly once per aggregation as contiguous 256 KB
fp8 pre-tiled blocks (mixed fp16-stationary x fp8-moving matmul is
exact for integer counts).  z1 and h stay SBUF-resident, so
MM1/AGG1/MM3 pipeline on per-tile dependencies.
"""

import sys

sys.path.insert(0, "/opt/trn_rl_repo")

import numpy as np

N_REAL = 20000
NCORES = 8
RBLK = 2500          # real nodes per core
BLK = 2560           # padded nodes per core (20 * 128)
NPAD = NCORES * BLK  # 20480
CIN = 256
CHID = 256
COUT = 128
P = 128
KT = NPAD // 512     # 40 k-tiles over nodes
NT = BLK // 512      # 5 n-tiles over a core's node block

_compiled = None


def _build_nc():
    import concourse.bass as bass  # noqa: F401
    import concourse.mybir as mybir
    import concourse.tile as tile
    from concourse import bacc
    from concourse.kernels import tile_matmul as tm
    from contextlib import ExitStack

    f16 = mybir.dt.float16
    f8 = mybir.dt.float8e4
    f32 = mybir.dt.float32
    Alu = mybir.AluOpType

    nc = bacc.Bacc("TRN2", target_bir_lowering=False, debug=False,
                   num_devices=NCORES)

    # External I/O, pre-tiled so every big DMA is one contiguous block.
    xT = nc.dram_tensor("xT", [KT, P, CIN // P, 512], f16,
                        kind="ExternalInput")
    W1 = nc.dram_tensor("W1", [P, CIN // P, CHID], f16, kind="ExternalInput")
    W2 = nc.dram_tensor("W2", [P, CHID // P, COUT], f16, kind="ExternalInput")
    Ab = nc.dram_tensor("Ab", [KT, NT, P, 4, 512], f8, kind="ExternalInput")
    sbc = nc.dram_tensor("sbc", [P, BLK], f32, kind="ExternalInput")
    dbc = nc.dram_tensor("dbc", [P, BLK], f32, kind="ExternalInput")
    dz1 = nc.dram_tensor("dz1", [P, NPAD // P], f32, kind="ExternalInput")
    dz2 = nc.dram_tensor("dz2", [P, BLK // P], f32, kind="ExternalInput")
    b1c = nc.dram_tensor("b1c", [P, CHID // P], f32, kind="ExternalInput")
    b2c = nc.dram_tensor("b2c", [P, COUT // P], f32, kind="ExternalInput")
    outT = nc.dram_tensor("outT", [P, 1, BLK], f32, kind="ExternalOutput")

    # Internal DRAM (collective buffers)
    z2b = nc.dram_tensor("z2b", [NT, P, 4, COUT], f16)
    z2g = nc.dram_tensor("z2g", [NT, NCORES, P, 4, COUT], f16,
                         addr_space="Shared")

    # Aggregation k-step q -> (core g, z2-tile t) interleaved t-major so
    # AGG2 can start on gather t=0 while later gathers are in flight.
    # Ab's tile axis is host-permuted to this order for both aggregations.
    def q_to_phys(q):
        t, g = divmod(q, NCORES)
        return g * NT + t  # physical global k-tile index

    with tile.TileContext(nc) as tc:
        with ExitStack() as octx:
            const = octx.enter_context(tc.tile_pool(name="const", bufs=1))
            s_sb = const.tile([P, BLK], f32)
            d_sb = const.tile([P, BLK], f32)
            dz1_sb = const.tile([P, NPAD // P], f32)
            dz2_sb = const.tile([P, BLK // P], f32)
            b1_sb = const.tile([P, CHID // P], f32)
            b2_sb = const.tile([P, COUT // P], f32)
            nc.sync.dma_start(s_sb[:], sbc[:])
            nc.sync.dma_start(d_sb[:], dbc[:])
            nc.sync.dma_start(dz1_sb[:], dz1[:])
            nc.sync.dma_start(dz2_sb[:], dz2[:])
            nc.sync.dma_start(b1_sb[:], b1c[:])
            nc.sync.dma_start(b2_sb[:], b2c[:])

            # z1 pool closes after AGG1 so its 10 MB is reusable for AGG2
            # prefetch buffers; opened last so pool release stays LIFO.
            h_pool = octx.enter_context(tc.tile_pool(name="hsb", bufs=NT + 1))
            z1_cm = tc.tile_pool(name="z1sb", bufs=KT + 1)
            z1_pool = z1_cm.__enter__()
            z1_tiles = {}
            h_tiles = {}

            # ---- Phase 1: z1' = dinv * (x @ W1), SBUF-resident -----------
            with ExitStack() as ctx:
                mm1_kxm = ctx.enter_context(
                    tc.tile_pool(name="mm1_kxm", bufs=6))
                mm1_kxn = ctx.enter_context(
                    tc.tile_pool(name="mm1_kxn", bufs=2))

                def xT_producer(nc, md):
                    t = mm1_kxm.tile([P, CIN // P, 512], f16, tag="xT")
                    nc.sync.dma_start(t[:], xT[md.m_tile_idx])
                    return t

                w1_producer, w1_shape = tm.dma_from_dram_kxn(mm1_kxn, W1[:])

                def z1_sink(nc, md):
                    t = z1_pool.tile([P, 4, CHID], f16, tag="z1")
                    z1_tiles[md.m_tile_idx] = t
                    return t

                def mm1_reducer(nc, psum, sbuf, md):
                    j = md.m_tile_idx * 4 + md.m_subtile_idx
                    nc.vector.tensor_scalar_mul(
                        sbuf[:, 0, :], psum[:], dz1_sb[:, j:j + 1])

                tm.composable_matmul_tile_kernel(
                    tc=tc,
                    kxm_shape=tm.ShapeInfo(pdims=((P, CIN // P),),
                                           fdims=(NPAD,)),
                    kxn_shape=w1_shape,
                    output_type=f16,
                    kxm_producer=xT_producer,
                    kxn_producer=w1_producer,
                    mxn_consumer=lambda nc, sbuf, md: None,
                    mxn_subtile_producer=z1_sink,
                    mxn_subtile_reducer=mm1_reducer,
                    psum_n_bufs=2,
                )

            # ---- Phase 2: hT = relu(dinv*contract(z1', C) + b1 (x) s) ----
            with ExitStack() as ctx:
                a1_kxn = ctx.enter_context(
                    tc.tile_pool(name="a1_kxn", bufs=16))
                a1_red = ctx.enter_context(tc.tile_pool(name="a1_red",
                                                        bufs=4))

                def a_producer_factory(pool, name):
                    def produce(nc, md):
                        t = pool.tile([P, 4, 512], f8, tag=name)
                        nc.sync.dma_start(t[:],
                                          Ab[md.k_tile_idx, md.n_tile_idx])
                        return t
                    return produce

                def h_sink(nc, md):
                    t = h_pool.tile([P, CHID // P, 512], f16, tag="h")
                    h_tiles[md.n_tile_idx] = t
                    return t

                def a1_reducer(nc, psum, sbuf, md):
                    n0 = md.n_tile_idx * md.n_tile
                    nsz = psum.shape[-1]
                    mi = md.m_subtile_idx
                    tmp = a1_red.tile([P, 512], f32, tag="a1t")
                    nc.vector.tensor_mul(tmp[:, :nsz], psum[:],
                                         d_sb[:, n0:n0 + nsz])
                    nc.vector.scalar_tensor_tensor(
                        tmp[:, :nsz], s_sb[:, n0:n0 + nsz],
                        b1_sb[:, mi:mi + 1], tmp[:, :nsz],
                        op0=Alu.mult, op1=Alu.add)
                    nc.vector.tensor_scalar_max(sbuf[:, 0, :], tmp[:, :nsz],
                                                0.0)

                tm.composable_matmul_tile_kernel(
                    tc=tc,
                    kxm_shape=tm.ShapeInfo(pdims=((P, NPAD // P),),
                                           fdims=(CHID,)),
                    kxn_shape=tm.ShapeInfo(pdims=((P, NPAD // P),),
                                           fdims=(BLK,)),
                    output_type=f16,
                    kxm_producer=lambda nc, md: z1_tiles[
                        q_to_phys(md.k_tile_idx)],
                    kxn_producer=a_producer_factory(a1_kxn, "a1A"),
                    mxn_consumer=lambda nc, sbuf, md: None,
                    mxn_subtile_producer=h_sink,
                    mxn_subtile_reducer=a1_reducer,
                    psum_n_bufs=3,
                )
            z1_cm.__exit__(None, None, None)

            # ---- Phase 3: z2' = dinv * (h_blk @ W2) ----------------------
            with ExitStack() as ctx:
                mm3_kxn = ctx.enter_context(
                    tc.tile_pool(name="mm3_kxn", bufs=2))
                w2_producer, w2_shape = tm.dma_from_dram_kxn(mm3_kxn, W2[:])

                def mm3_reducer(nc, psum, sbuf, md):
                    j = md.m_tile_idx * 4 + md.m_subtile_idx
                    nc.vector.tensor_scalar_mul(
                        sbuf[:, 0, :], psum[:], dz2_sb[:, j:j + 1])

                def z2_consumer(nc, sbuf, md):
                    nc.sync.dma_start(z2b[md.m_tile_idx], sbuf[:])

                tm.composable_matmul_tile_kernel(
                    tc=tc,
                    kxm_shape=tm.ShapeInfo(pdims=((P, CHID // P),),
                                           fdims=(BLK,)),
                    kxn_shape=w2_shape,
                    output_type=f16,
                    kxm_producer=lambda nc, md: h_tiles[md.m_tile_idx],
                    kxn_producer=w2_producer,
                    mxn_consumer=z2_consumer,
                    mxn_subtile_reducer=mm3_reducer,
                    psum_n_bufs=2,
                )

            # ---- Phase 4: per-tile AllGather of z2' blocks ---------------
            for t in range(NT):
                nc.gpsimd.collective_compute(
                    "AllGather",
                    mybir.AluOpType.bypass,
                    ins=[z2b[t]],
                    outs=[z2g[t]],
                    replica_groups=[list(range(NCORES))],
                )

            # ---- Phase 5: outT = dinv*contract(z2', C) + b2 (x) s --------
            # Hand-rolled k-outer loop: one PSUM bank per n-tile, so the
            # first gathered z2 tile starts compute while later gathers
            # are still in flight.
            with ExitStack() as ctx:
                a2_kxm = ctx.enter_context(
                    tc.tile_pool(name="a2_kxm", bufs=8))
                a2_kxn = ctx.enter_context(
                    tc.tile_pool(name="a2_kxn", bufs=32))
                a2_red = ctx.enter_context(tc.tile_pool(name="a2_red",
                                                        bufs=8))
                a2_ps = ctx.enter_context(
                    tc.tile_pool(name="a2_ps", bufs=1, space="PSUM"))

                psums = [a2_ps.tile([P, 512], f32, name=f"a2ps{n}")
                         for n in range(NT)]
                for q in range(KT):
                    t, g = divmod(q, NCORES)
                    zt = a2_kxm.tile([P, 4, COUT], f16, tag="z2kxm")
                    nc.sync.dma_start(zt[:], z2g[t, g])
                    ats = []
                    for n in range(NT):
                        at = a2_kxn.tile([P, 4, 512], f8, tag="a2A")
                        nc.sync.dma_start(at[:], Ab[q, n])
                        ats.append(at)
                    for ks in range(4):
                        for n in range(NT):
                            nc.tensor.matmul(
                                psums[n][:], zt[:, ks], ats[n][:, ks],
                                start=(q == 0 and ks == 0),
                                stop=(q == KT - 1 and ks == 3))

                for n in range(NT):
                    n0 = n * 512
                    tmp = a2_red.tile([P, 512], f32, tag="a2t")
                    osb = a2_red.tile([P, 512], f32, tag="a2o")
                    nc.vector.tensor_mul(tmp[:], psums[n][:],
                                         d_sb[:, n0:n0 + 512])
                    nc.vector.scalar_tensor_tensor(
                        osb[:], s_sb[:, n0:n0 + 512],
                        b2_sb[:, 0:1], tmp[:],
                        op0=Alu.mult, op1=Alu.add)
                    nc.sync.dma_start(outT[:, 0, n0:n0 + 512], osb[:])

    nc.compile()
    return nc


def _preprocess(x, edge_index, W1, b1, W2, b2):
    import ml_dtypes

    x = np.asarray(x, dtype=np.float32)
    edge_index = np.asarray(edge_index)
    W1 = np.asarray(W1, dtype=np.float32)
    b1 = np.asarray(b1, dtype=np.float32)
    W2 = np.asarray(W2, dtype=np.float32)
    b2 = np.asarray(b2, dtype=np.float32)

    row = edge_index[0].astype(np.int64)
    col = edge_index[1].astype(np.int64)

    deg = np.bincount(col, minlength=N_REAL).astype(np.float32) + 1.0
    dinv = 1.0 / np.sqrt(deg)

    idx = np.arange(N_REAL, dtype=np.int64)
    pad_id = (idx // RBLK) * BLK + idx % RBLK  # real -> padded node id

    # Dense count matrix, transposed: CT[src, dst] = A[dst, src] + I
    CT = np.zeros((NPAD, NPAD), dtype=np.uint8)
    np.add.at(CT, (pad_id[row], pad_id[col]), 1)
    CT[pad_id, pad_id] += 1
    assert CT.max() <= 16, "count exceeds exact fp8e4m3 integer range"

    # s[c] = sum_r A_hat[c, r]; dinv at padded positions -> 0
    s_real = dinv * (np.bincount(col, weights=dinv[row],
                                 minlength=N_REAL).astype(np.float32) + dinv)
    s_pad = np.zeros(NPAD, dtype=np.float32)
    s_pad[pad_id] = s_real
    dinv_pad = np.zeros(NPAD, dtype=np.float32)
    dinv_pad[pad_id] = dinv

    x_pad = np.zeros((NPAD, CIN), dtype=np.float16)
    x_pad[pad_id] = x.astype(np.float16)
    # xT tile layout: [mt][p][kt][ml] = x_pad[mt*512 + ml, kt*128 + p]
    xT_t = np.ascontiguousarray(
        x_pad.reshape(KT, 512, CIN // P, P).transpose(0, 3, 2, 1))

    W1_t = np.ascontiguousarray(
        W1.astype(np.float16).reshape(CIN // P, P, CHID).transpose(1, 0, 2))
    W2_t = np.ascontiguousarray(
        W2.astype(np.float16).reshape(CHID // P, P, COUT).transpose(1, 0, 2))
    b1_t = np.ascontiguousarray(b1.reshape(CHID // P, P).T)
    b2_t = np.ascontiguousarray(b2.reshape(COUT // P, P).T)
    # node-major per-partition dinv: [p][j] = dinv_pad[j*128 + p]
    dz1_t = np.ascontiguousarray(dinv_pad.reshape(NPAD // P, P).T)

    in_maps = []
    for g in range(NCORES):
        C_g = CT[:, g * BLK:(g + 1) * BLK]
        # [kt][nt][p][s][n] = C_g[kt*512 + s*128 + p, nt*512 + n],
        # then permute the kt axis into the device's q-order
        # (q -> physical kt = (q % NCORES) * NT + q // NCORES).
        perm = [(q % NCORES) * NT + q // NCORES for q in range(KT)]
        A_t = np.ascontiguousarray(
            C_g.reshape(KT, 4, P, NT, 512).transpose(0, 3, 2, 1, 4)[perm]
        ).astype(ml_dtypes.float8_e4m3)
        s_loc = s_pad[g * BLK:(g + 1) * BLK]
        d_loc = dinv_pad[g * BLK:(g + 1) * BLK]
        s_b = np.ascontiguousarray(
            np.broadcast_to(s_loc, (P, BLK))).astype(np.float32)
        d_b = np.ascontiguousarray(
            np.broadcast_to(d_loc, (P, BLK))).astype(np.float32)
        dz2_t = np.ascontiguousarray(d_loc.reshape(BLK // P, P).T)
        in_maps.append(dict(xT=xT_t, W1=W1_t, W2=W2_t, Ab=A_t,
                            sbc=s_b, dbc=d_b, dz1=dz1_t, dz2=dz2_t,
                            b1c=b1_t, b2c=b2_t))
    return in_maps


def _run(inputs, trace=False):
    global _compiled
    if _compiled is None:
        _compiled = _build_nc()
    nc = _compiled
    from concourse.bass_utils import run_bass_kernel_spmd

    in_maps = _preprocess(**inputs)
    res = run_bass_kernel_spmd(nc, in_maps, list(range(NCORES)), trace=trace)
    out = np.empty((N_REAL, COUT), dtype=np.float32)
    for g in range(NCORES):
        out[g * RBLK:(g + 1) * RBLK] = res.results[g]["outT"][:, 0, :RBLK].T
    return out, res


def kernel(**inputs) -> np.ndarray:
    out, _ = _run(inputs, trace=False)
    return out


# revision 19
# speedup vs baseline: 1.0208x; 1.0208x over previous
"""GCN encoder (2-layer) on 8 Trainium2 NeuronCores.

Strategy: the GCN layer out = A_hat @ (x @ W + b) with
A_hat = D^-1/2 (A + I) D^-1/2 is cast as dense matmuls on the tensor
engine.  The count matrix C = A + I (20000x20000, 0.16% dense) is
materialized host-side in fp8 (small integer counts -> exact) from the
edge list (index-only preprocessing), node-partitioned column-blocks
across the 8 cores.  The D^-1/2 factors fold into cheap per-row /
per-column scalings at PSUM eviction (relu commutes with the positive
scaling), and biases fold in as rank-1 updates b (x) s with
s[c] = sum_r A_hat[c, r] host-computed.  Per core:

  MM1   z1' = dinv * (x @ W1)             (replicated, node-major)
  AGG1  hT  = relu(dinv * contract_k(z1', C_blk) + b1 (x) s)
  MM3   z2' = dinv * (h_blk @ W2)         (node-major local block)
  AllGather z2' blocks across cores
  AGG2  outT = dinv * contract_k(z2', C_blk) + b2 (x) s

C_blk is the kxn operand of both aggregations with M_TILES == 1, so it
streams from HBM exactly once per aggregation as contiguous 256 KB
fp8 pre-tiled blocks (mixed fp16-stationary x fp8-moving matmul is
exact for integer counts).  z1 and h stay SBUF-resident, so
MM1/AGG1/MM3 pipeline on per-tile dependencies.
"""

import sys

sys.path.insert(0, "/opt/trn_rl_repo")

import numpy as np

N_REAL = 20000
NCORES = 8
RBLK = 2500          # real nodes per core
BLK = 2560           # padded nodes per core (20 * 128)
NPAD = NCORES * BLK  # 20480
CIN = 256
CHID = 256
COUT = 128
P = 128
KT = NPAD // 512     # 40 k-tiles over nodes
NT = BLK // 512      # 5 n-tiles over a core's node block

_compiled = None


def _build_nc():
    import concourse.bass as bass  # noqa: F401
    import concourse.mybir as mybir
    import concourse.tile as tile
    from concourse import bacc
    from concourse.kernels import tile_matmul as tm
    from contextlib import ExitStack

    f16 = mybir.dt.float16
    f8 = mybir.dt.float8e4
    f32 = mybir.dt.float32
    Alu = mybir.AluOpType

    nc = bacc.Bacc("TRN2", target_bir_lowering=False, debug=False,
                   num_devices=NCORES)

    # External I/O, pre-tiled so every big DMA is one contiguous block.
    xT = nc.dram_tensor("xT", [KT, P, CIN // P, 512], f16,
                        kind="ExternalInput")
    W1 = nc.dram_tensor("W1", [P, CIN // P, CHID], f16, kind="ExternalInput")
    W2 = nc.dram_tensor("W2", [P, CHID // P, COUT], f16, kind="ExternalInput")
    Ab = nc.dram_tensor("Ab", [KT, NT, P, 4, 512], f8, kind="ExternalInput")
    sbc = nc.dram_tensor("sbc", [P, BLK], f32, kind="ExternalInput")
    dbc = nc.dram_tensor("dbc", [P, BLK], f32, kind="ExternalInput")
    dz1 = nc.dram_tensor("dz1", [P, NPAD // P], f32, kind="ExternalInput")
    dz2 = nc.dram_tensor("dz2", [P, BLK // P], f32, kind="ExternalInput")
    b1c = nc.dram_tensor("b1c", [P, CHID // P], f32, kind="ExternalInput")
    b2c = nc.dram_tensor("b2c", [P, COUT // P], f32, kind="ExternalInput")
    outT = nc.dram_tensor("outT", [P, 1, BLK], f32, kind="ExternalOutput")

    # Internal DRAM (collective buffers).  Two gather groups (t=0..2 and
    # t=3..4) in separate tensors so the first gather's dependencies are
    # only the first three z2 tiles and it can run during AGG1's tail.
    NT_A = 3
    z2bA = nc.dram_tensor("z2bA", [NT_A, P, 4, COUT], f16)
    z2bB = nc.dram_tensor("z2bB", [NT - NT_A, P, 4, COUT], f16)
    z2gA = nc.dram_tensor("z2gA", [NCORES, NT_A, P, 4, COUT], f16,
                          addr_space="Shared")
    z2gB = nc.dram_tensor("z2gB", [NCORES, NT - NT_A, P, 4, COUT], f16,
                          addr_space="Shared")

    # Aggregation k-step q -> (core g, z2-tile t) interleaved t-major so
    # AGG2 can start on gather t=0 while later gathers are in flight.
    # Ab's tile axis is host-permuted to this order for both aggregations.
    def q_to_phys(q):
        t, g = divmod(q, NCORES)
        return g * NT + t  # physical global k-tile index

    with tile.TileContext(nc) as tc:
        with ExitStack() as octx:
            const = octx.enter_context(tc.tile_pool(name="const", bufs=1))
            s_sb = const.tile([P, BLK], f32)
            d_sb = const.tile([P, BLK], f32)
            dz1_sb = const.tile([P, NPAD // P], f32)
            dz2_sb = const.tile([P, BLK // P], f32)
            b1_sb = const.tile([P, CHID // P], f32)
            b2_sb = const.tile([P, COUT // P], f32)
            nc.sync.dma_start(s_sb[:], sbc[:])
            nc.sync.dma_start(d_sb[:], dbc[:])
            nc.sync.dma_start(dz1_sb[:], dz1[:])
            nc.sync.dma_start(dz2_sb[:], dz2[:])
            nc.sync.dma_start(b1_sb[:], b1c[:])
            nc.sync.dma_start(b2_sb[:], b2c[:])

            # z1 pool closes after AGG1 so its 10 MB is reusable for AGG2
            # prefetch buffers; opened last so pool release stays LIFO.
            h_pool = octx.enter_context(tc.tile_pool(name="hsb", bufs=NT + 1))
            z1_cm = tc.tile_pool(name="z1sb", bufs=KT + 1)
            z1_pool = z1_cm.__enter__()
            z1_tiles = {}
            h_tiles = {}

            # ---- Phase 1: z1' = dinv * (x @ W1), SBUF-resident -----------
            with ExitStack() as ctx:
                mm1_kxm = ctx.enter_context(
                    tc.tile_pool(name="mm1_kxm", bufs=6))
                mm1_kxn = ctx.enter_context(
                    tc.tile_pool(name="mm1_kxn", bufs=2))

                def xT_producer(nc, md):
                    t = mm1_kxm.tile([P, CIN // P, 512], f16, tag="xT")
                    nc.sync.dma_start(t[:], xT[md.m_tile_idx])
                    return t

                w1_producer, w1_shape = tm.dma_from_dram_kxn(mm1_kxn, W1[:])

                def z1_sink(nc, md):
                    t = z1_pool.tile([P, 4, CHID], f16, tag="z1")
                    z1_tiles[md.m_tile_idx] = t
                    return t

                def mm1_reducer(nc, psum, sbuf, md):
                    j = md.m_tile_idx * 4 + md.m_subtile_idx
                    nc.vector.tensor_scalar_mul(
                        sbuf[:, 0, :], psum[:], dz1_sb[:, j:j + 1])

                tm.composable_matmul_tile_kernel(
                    tc=tc,
                    kxm_shape=tm.ShapeInfo(pdims=((P, CIN // P),),
                                           fdims=(NPAD,)),
                    kxn_shape=w1_shape,
                    output_type=f16,
                    kxm_producer=xT_producer,
                    kxn_producer=w1_producer,
                    mxn_consumer=lambda nc, sbuf, md: None,
                    mxn_subtile_producer=z1_sink,
                    mxn_subtile_reducer=mm1_reducer,
                    psum_n_bufs=2,
                )

            # ---- Phase 2: hT = relu(dinv*contract(z1', C) + b1 (x) s) ----
            with ExitStack() as ctx:
                a1_kxn = ctx.enter_context(
                    tc.tile_pool(name="a1_kxn", bufs=16))
                a1_red = ctx.enter_context(tc.tile_pool(name="a1_red",
                                                        bufs=4))

                def a_producer_factory(pool, name):
                    def produce(nc, md):
                        t = pool.tile([P, 4, 512], f8, tag=name)
                        nc.sync.dma_start(t[:],
                                          Ab[md.k_tile_idx, md.n_tile_idx])
                        return t
                    return produce

                def h_sink(nc, md):
                    t = h_pool.tile([P, CHID // P, 512], f16, tag="h")
                    h_tiles[md.n_tile_idx] = t
                    return t

                def a1_reducer(nc, psum, sbuf, md):
                    n0 = md.n_tile_idx * md.n_tile
                    nsz = psum.shape[-1]
                    mi = md.m_subtile_idx
                    tmp = a1_red.tile([P, 512], f32, tag="a1t")
                    nc.vector.tensor_mul(tmp[:, :nsz], psum[:],
                                         d_sb[:, n0:n0 + nsz])
                    nc.vector.scalar_tensor_tensor(
                        tmp[:, :nsz], s_sb[:, n0:n0 + nsz],
                        b1_sb[:, mi:mi + 1], tmp[:, :nsz],
                        op0=Alu.mult, op1=Alu.add)
                    nc.vector.tensor_scalar_max(sbuf[:, 0, :], tmp[:, :nsz],
                                                0.0)

                tm.composable_matmul_tile_kernel(
                    tc=tc,
                    kxm_shape=tm.ShapeInfo(pdims=((P, NPAD // P),),
                                           fdims=(CHID,)),
                    kxn_shape=tm.ShapeInfo(pdims=((P, NPAD // P),),
                                           fdims=(BLK,)),
                    output_type=f16,
                    kxm_producer=lambda nc, md: z1_tiles[
                        q_to_phys(md.k_tile_idx)],
                    kxn_producer=a_producer_factory(a1_kxn, "a1A"),
                    mxn_consumer=lambda nc, sbuf, md: None,
                    mxn_subtile_producer=h_sink,
                    mxn_subtile_reducer=a1_reducer,
                    psum_n_bufs=3,
                )
            z1_cm.__exit__(None, None, None)

            # ---- Phase 3: z2' = dinv * (h_blk @ W2) ----------------------
            with ExitStack() as ctx:
                mm3_kxn = ctx.enter_context(
                    tc.tile_pool(name="mm3_kxn", bufs=2))
                w2_producer, w2_shape = tm.dma_from_dram_kxn(mm3_kxn, W2[:])

                def mm3_reducer(nc, psum, sbuf, md):
                    j = md.m_tile_idx * 4 + md.m_subtile_idx
                    nc.vector.tensor_scalar_mul(
                        sbuf[:, 0, :], psum[:], dz2_sb[:, j:j + 1])

                def z2_consumer(nc, sbuf, md):
                    t = md.m_tile_idx
                    dst = z2bA[t] if t < NT_A else z2bB[t - NT_A]
                    nc.sync.dma_start(dst, sbuf[:])

                tm.composable_matmul_tile_kernel(
                    tc=tc,
                    kxm_shape=tm.ShapeInfo(pdims=((P, CHID // P),),
                                           fdims=(BLK,)),
                    kxn_shape=w2_shape,
                    output_type=f16,
                    kxm_producer=lambda nc, md: h_tiles[md.m_tile_idx],
                    kxn_producer=w2_producer,
                    mxn_consumer=z2_consumer,
                    mxn_subtile_reducer=mm3_reducer,
                    psum_n_bufs=2,
                )

            # ---- Phase 4: two AllGathers of z2' blocks -------------------
            nc.gpsimd.collective_compute(
                "AllGather",
                mybir.AluOpType.bypass,
                ins=[z2bA[:]],
                outs=[z2gA[:]],
                replica_groups=[list(range(NCORES))],
            )
            nc.gpsimd.collective_compute(
                "AllGather",
                mybir.AluOpType.bypass,
                ins=[z2bB[:]],
                outs=[z2gB[:]],
                replica_groups=[list(range(NCORES))],
            )

            # ---- Phase 5: outT = dinv*contract(z2', C) + b2 (x) s --------
            # Hand-rolled k-outer loop: one PSUM bank per n-tile, so the
            # first gathered z2 tile starts compute while later gathers
            # are still in flight.
            with ExitStack() as ctx:
                a2_kxm = ctx.enter_context(
                    tc.tile_pool(name="a2_kxm", bufs=8))
                a2_kxn = ctx.enter_context(
                    tc.tile_pool(name="a2_kxn", bufs=32))
                a2_red = ctx.enter_context(tc.tile_pool(name="a2_red",
                                                        bufs=8))
                a2_ps = ctx.enter_context(
                    tc.tile_pool(name="a2_ps", bufs=1, space="PSUM"))

                psums = [a2_ps.tile([P, 512], f32, name=f"a2ps{n}")
                         for n in range(NT)]
                for q in range(KT):
                    t, g = divmod(q, NCORES)
                    src = z2gA[g, t] if t < NT_A else z2gB[g, t - NT_A]
                    zt = a2_kxm.tile([P, 4, COUT], f16, tag="z2kxm")
                    nc.sync.dma_start(zt[:], src)
                    ats = []
                    for n in range(NT):
                        at = a2_kxn.tile([P, 4, 512], f8, tag="a2A")
                        nc.sync.dma_start(at[:], Ab[q, n])
                        ats.append(at)
                    for ks in range(4):
                        for n in range(NT):
                            nc.tensor.matmul(
                                psums[n][:], zt[:, ks], ats[n][:, ks],
                                start=(q == 0 and ks == 0),
                                stop=(q == KT - 1 and ks == 3))

                for n in range(NT):
                    n0 = n * 512
                    tmp = a2_red.tile([P, 512], f32, tag="a2t")
                    osb = a2_red.tile([P, 512], f32, tag="a2o")
                    nc.vector.tensor_mul(tmp[:], psums[n][:],
                                         d_sb[:, n0:n0 + 512])
                    nc.vector.scalar_tensor_tensor(
                        osb[:], s_sb[:, n0:n0 + 512],
                        b2_sb[:, 0:1], tmp[:],
                        op0=Alu.mult, op1=Alu.add)
                    nc.sync.dma_start(outT[:, 0, n0:n0 + 512], osb[:])

    nc.compile()
    return nc


def _preprocess(x, edge_index, W1, b1, W2, b2):
    import ml_dtypes

    x = np.asarray(x, dtype=np.float32)
    edge_index = np.asarray(edge_index)
    W1 = np.asarray(W1, dtype=np.float32)
    b1 = np.asarray(b1, dtype=np.float32)
    W2 = np.asarray(W2, dtype=np.float32)
    b2 = np.asarray(b2, dtype=np.float32)

    row = edge_index[0].astype(np.int64)
    col = edge_index[1].astype(np.int64)

    deg = np.bincount(col, minlength=N_REAL).astype(np.float32) + 1.0
    dinv = 1.0 / np.sqrt(deg)

    idx = np.arange(N_REAL, dtype=np.int64)
    pad_id = (idx // RBLK) * BLK + idx % RBLK  # real -> padded node id

    # Dense count matrix, transposed: CT[src, dst] = A[dst, src] + I
    CT = np.zeros((NPAD, NPAD), dtype=np.uint8)
    np.add.at(CT, (pad_id[row], pad_id[col]), 1)
    CT[pad_id, pad_id] += 1
    assert CT.max() <= 16, "count exceeds exact fp8e4m3 integer range"

    # s[c] = sum_r A_hat[c, r]; dinv at padded positions -> 0
    s_real = dinv * (np.bincount(col, weights=dinv[row],
                                 minlength=N_REAL).astype(np.float32) + dinv)
    s_pad = np.zeros(NPAD, dtype=np.float32)
    s_pad[pad_id] = s_real
    dinv_pad = np.zeros(NPAD, dtype=np.float32)
    dinv_pad[pad_id] = dinv

    x_pad = np.zeros((NPAD, CIN), dtype=np.float16)
    x_pad[pad_id] = x.astype(np.float16)
    # xT tile layout: [mt][p][kt][ml] = x_pad[mt*512 + ml, kt*128 + p]
    xT_t = np.ascontiguousarray(
        x_pad.reshape(KT, 512, CIN // P, P).transpose(0, 3, 2, 1))

    W1_t = np.ascontiguousarray(
        W1.astype(np.float16).reshape(CIN // P, P, CHID).transpose(1, 0, 2))
    W2_t = np.ascontiguousarray(
        W2.astype(np.float16).reshape(CHID // P, P, COUT).transpose(1, 0, 2))
    b1_t = np.ascontiguousarray(b1.reshape(CHID // P, P).T)
    b2_t = np.ascontiguousarray(b2.reshape(COUT // P, P).T)
    # node-major per-partition dinv: [p][j] = dinv_pad[j*128 + p]
    dz1_t = np.ascontiguousarray(dinv_pad.reshape(NPAD // P, P).T)

    in_maps = []
    for g in range(NCORES):
        C_g = CT[:, g * BLK:(g + 1) * BLK]
        # [kt][nt][p][s][n] = C_g[kt*512 + s*128 + p, nt*512 + n],
        # then permute the kt axis into the device's q-order
        # (q -> physical kt = (q % NCORES) * NT + q // NCORES).
        perm = [(q % NCORES) * NT + q // NCORES for q in range(KT)]
        A_t = np.ascontiguousarray(
            C_g.reshape(KT, 4, P, NT, 512).transpose(0, 3, 2, 1, 4)[perm]
        ).astype(ml_dtypes.float8_e4m3)
        s_loc = s_pad[g * BLK:(g + 1) * BLK]
        d_loc = dinv_pad[g * BLK:(g + 1) * BLK]
        s_b = np.ascontiguousarray(
            np.broadcast_to(s_loc, (P, BLK))).astype(np.float32)
        d_b = np.ascontiguousarray(
            np.broadcast_to(d_loc, (P, BLK))).astype(np.float32)
        dz2_t = np.ascontiguousarray(d_loc.reshape(BLK // P, P).T)
        in_maps.append(dict(xT=xT_t, W1=W1_t, W2=W2_t, Ab=A_t,
                            sbc=s_b, dbc=d_b, dz1=dz1_t, dz2=dz2_t,
                            b1c=b1_t, b2c=b2_t))
    return in_maps


def _run(inputs, trace=False):
    global _compiled
    if _compiled is None:
        _compiled = _build_nc()
    nc = _compiled
    from concourse.bass_utils import run_bass_kernel_spmd

    in_maps = _preprocess(**inputs)
    res = run_bass_kernel_spmd(nc, in_maps, list(range(NCORES)), trace=trace)
    out = np.empty((N_REAL, COUT), dtype=np.float32)
    for g in range(NCORES):
        out[g * RBLK:(g + 1) * RBLK] = res.results[g]["outT"][:, 0, :RBLK].T
    return out, res


def kernel(**inputs) -> np.ndarray:
    out, _ = _run(inputs, trace=False)
    return out


# revision 20
# speedup vs baseline: 1.0685x; 1.0467x over previous
"""GCN encoder (2-layer) on 8 Trainium2 NeuronCores.

Strategy: the GCN layer out = A_hat @ (x @ W + b) with
A_hat = D^-1/2 (A + I) D^-1/2 is cast as dense matmuls on the tensor
engine.  The count matrix C = A + I (20000x20000, 0.16% dense) is
materialized host-side in fp8 (small integer counts -> exact) from the
edge list (index-only preprocessing), node-partitioned column-blocks
across the 8 cores.  The D^-1/2 factors fold into cheap per-row /
per-column scalings at PSUM eviction (relu commutes with the positive
scaling), and biases fold in as rank-1 updates b (x) s with
s[c] = sum_r A_hat[c, r] host-computed.  Per core:

  MM1   z1' = dinv * (x @ W1)             (replicated, node-major)
  AGG1  hT  = relu(dinv * contract_k(z1', C_blk) + b1 (x) s)
  MM3   z2' = dinv * (h_blk @ W2)         (node-major local block)
  AllGather z2' blocks across cores
  AGG2  outT = dinv * contract_k(z2', C_blk) + b2 (x) s

C_blk is the kxn operand of both aggregations with M_TILES == 1, so it
streams from HBM exactly once per aggregation as contiguous 256 KB
fp8 pre-tiled blocks (mixed fp16-stationary x fp8-moving matmul is
exact for integer counts).  z1 and h stay SBUF-resident, so
MM1/AGG1/MM3 pipeline on per-tile dependencies.
"""

import sys

sys.path.insert(0, "/opt/trn_rl_repo")

import numpy as np

N_REAL = 20000
NCORES = 8
RBLK = 2500          # real nodes per core
BLK = 2560           # padded nodes per core (20 * 128)
NPAD = NCORES * BLK  # 20480
CIN = 256
CHID = 256
COUT = 128
P = 128
KT = NPAD // 512     # 40 k-tiles over nodes
NT = BLK // 512      # 5 n-tiles over a core's node block

_compiled = None


def _build_nc():
    import concourse.bass as bass  # noqa: F401
    import concourse.mybir as mybir
    import concourse.tile as tile
    from concourse import bacc
    from concourse.kernels import tile_matmul as tm
    from contextlib import ExitStack

    f16 = mybir.dt.float16
    f8 = mybir.dt.float8e4
    f32 = mybir.dt.float32
    Alu = mybir.AluOpType

    nc = bacc.Bacc("TRN2", target_bir_lowering=False, debug=False,
                   num_devices=NCORES)

    # External I/O, pre-tiled so every big DMA is one contiguous block.
    xT = nc.dram_tensor("xT", [KT, P, CIN // P, 512], f16,
                        kind="ExternalInput")
    W1 = nc.dram_tensor("W1", [P, CIN // P, CHID], f16, kind="ExternalInput")
    W2 = nc.dram_tensor("W2", [P, CHID // P, COUT], f16, kind="ExternalInput")
    Ab = nc.dram_tensor("Ab", [KT, NT, P, 4, 512], f8, kind="ExternalInput")
    sbc = nc.dram_tensor("sbc", [P, BLK], f32, kind="ExternalInput")
    dbc = nc.dram_tensor("dbc", [P, BLK], f32, kind="ExternalInput")
    dz1 = nc.dram_tensor("dz1", [P, NPAD // P], f32, kind="ExternalInput")
    dz2 = nc.dram_tensor("dz2", [P, BLK // P], f32, kind="ExternalInput")
    b1c = nc.dram_tensor("b1c", [P, CHID // P], f32, kind="ExternalInput")
    b2c = nc.dram_tensor("b2c", [P, COUT // P], f32, kind="ExternalInput")
    outT = nc.dram_tensor("outT", [P, 1, BLK], f32, kind="ExternalOutput")

    # Internal DRAM (collective buffers).  Two gather groups (t=0..2 and
    # t=3..4) in separate tensors so the first gather's dependencies are
    # only the first three z2 tiles and it can run during AGG1's tail.
    NT_A = 3
    z2bA = nc.dram_tensor("z2bA", [NT_A, P, 4, COUT], f16)
    z2bB = nc.dram_tensor("z2bB", [NT - NT_A, P, 4, COUT], f16)
    z2gA = nc.dram_tensor("z2gA", [NCORES, NT_A, P, 4, COUT], f16,
                          addr_space="Shared")
    z2gB = nc.dram_tensor("z2gB", [NCORES, NT - NT_A, P, 4, COUT], f16,
                          addr_space="Shared")

    # Aggregation k-step q -> (core g, z2-tile t) interleaved t-major so
    # AGG2 can start on gather t=0 while later gathers are in flight.
    # Ab's tile axis is host-permuted to this order for both aggregations.
    def q_to_phys(q):
        t, g = divmod(q, NCORES)
        return g * NT + t  # physical global k-tile index

    with tile.TileContext(nc) as tc:
        with ExitStack() as octx:
            const = octx.enter_context(tc.tile_pool(name="const", bufs=1))
            s_sb = const.tile([P, BLK], f32)
            d_sb = const.tile([P, BLK], f32)
            dz1_sb = const.tile([P, NPAD // P], f32)
            dz2_sb = const.tile([P, BLK // P], f32)
            b1_sb = const.tile([P, CHID // P], f32)
            b2_sb = const.tile([P, COUT // P], f32)
            nc.sync.dma_start(s_sb[:], sbc[:])
            nc.sync.dma_start(d_sb[:], dbc[:])
            nc.sync.dma_start(dz1_sb[:], dz1[:])
            nc.sync.dma_start(dz2_sb[:], dz2[:])
            nc.sync.dma_start(b1_sb[:], b1c[:])
            nc.sync.dma_start(b2_sb[:], b2c[:])

            # z1 pool closes after AGG1 so its 10 MB is reusable for AGG2
            # prefetch buffers; opened last so pool release stays LIFO.
            h_pool = octx.enter_context(tc.tile_pool(name="hsb", bufs=NT + 1))
            z1_cm = tc.tile_pool(name="z1sb", bufs=KT + 1)
            z1_pool = z1_cm.__enter__()
            z1_tiles = {}
            h_tiles = {}

            # ---- Phase 1: z1' = dinv * (x @ W1), SBUF-resident -----------
            with ExitStack() as ctx:
                mm1_kxm = ctx.enter_context(
                    tc.tile_pool(name="mm1_kxm", bufs=6))
                mm1_kxn = ctx.enter_context(
                    tc.tile_pool(name="mm1_kxn", bufs=2))

                def xT_producer(nc, md):
                    t = mm1_kxm.tile([P, CIN // P, 512], f16, tag="xT")
                    nc.sync.dma_start(t[:], xT[md.m_tile_idx])
                    return t

                w1_producer, w1_shape = tm.dma_from_dram_kxn(mm1_kxn, W1[:])

                def z1_sink(nc, md):
                    t = z1_pool.tile([P, 4, CHID], f16, tag="z1")
                    z1_tiles[md.m_tile_idx] = t
                    return t

                def mm1_reducer(nc, psum, sbuf, md):
                    j = md.m_tile_idx * 4 + md.m_subtile_idx
                    nc.vector.tensor_scalar_mul(
                        sbuf[:, 0, :], psum[:], dz1_sb[:, j:j + 1])

                tm.composable_matmul_tile_kernel(
                    tc=tc,
                    kxm_shape=tm.ShapeInfo(pdims=((P, CIN // P),),
                                           fdims=(NPAD,)),
                    kxn_shape=w1_shape,
                    output_type=f16,
                    kxm_producer=xT_producer,
                    kxn_producer=w1_producer,
                    mxn_consumer=lambda nc, sbuf, md: None,
                    mxn_subtile_producer=z1_sink,
                    mxn_subtile_reducer=mm1_reducer,
                    psum_n_bufs=2,
                )

            # ---- Phases 2-4 fused: AGG1 + MM3 + gathers ------------------
            # Hand-rolled so MM3's small matmul block for node-tile nt is
            # emitted right after AGG1's n_outer=nt (engines execute their
            # streams in order), and gather A fires mid-AGG1.
            with ExitStack() as ctx:
                a1_kxn = ctx.enter_context(
                    tc.tile_pool(name="a1_kxn", bufs=16))
                a1_red = ctx.enter_context(tc.tile_pool(name="a1_red",
                                                        bufs=4))
                a1_ps = ctx.enter_context(
                    tc.tile_pool(name="a1_ps", bufs=2, space="PSUM"))
                mm3_ps = ctx.enter_context(
                    tc.tile_pool(name="mm3_ps", bufs=1, space="PSUM"))
                mm3_sb = ctx.enter_context(tc.tile_pool(name="mm3_sb",
                                                        bufs=2))
                w2_sb = const.tile([P, CHID // P, COUT], f16)
                nc.sync.dma_start(w2_sb[:], W2[:])

                for nt in range(NT):
                    n0 = nt * 512
                    psums = [a1_ps.tile([P, 512], f32, name=f"a1ps{m}")
                             for m in range(2)]
                    for q in range(KT):
                        at = a1_kxn.tile([P, 4, 512], f8, tag="a1A")
                        nc.sync.dma_start(at[:], Ab[q, nt])
                        z1t = z1_tiles[q_to_phys(q)]
                        for ks in range(4):
                            for m in range(2):
                                nc.tensor.matmul(
                                    psums[m][:],
                                    z1t[:, ks, m * P:(m + 1) * P],
                                    at[:, ks],
                                    start=(q == 0 and ks == 0),
                                    stop=(q == KT - 1 and ks == 3))
                    # evict: h = relu(dinv * psum + b1 (x) s), fp16
                    ht = h_pool.tile([P, CHID // P, 512], f16, tag="h")
                    h_tiles[nt] = ht
                    for m in range(2):
                        tmp = a1_red.tile([P, 512], f32, tag="a1t")
                        nc.vector.tensor_mul(tmp[:], psums[m][:],
                                             d_sb[:, n0:n0 + 512])
                        nc.vector.scalar_tensor_tensor(
                            tmp[:], s_sb[:, n0:n0 + 512],
                            b1_sb[:, m:m + 1], tmp[:],
                            op0=Alu.mult, op1=Alu.add)
                        nc.vector.tensor_scalar_max(ht[:, m, :], tmp[:], 0.0)

                    # MM3 block: z2'[nt] = dinv * (h[nt] @ W2)
                    zsb = mm3_sb.tile([P, 4, COUT], f16, tag="z2sb")
                    for mi in range(4):
                        ps3 = mm3_ps.tile([P, COUT], f32, name=f"m3ps{mi}")
                        for ks in range(2):
                            nc.tensor.matmul(
                                ps3[:], ht[:, ks, mi * P:(mi + 1) * P],
                                w2_sb[:, ks],
                                start=(ks == 0), stop=(ks == 1))
                        nc.vector.tensor_scalar_mul(
                            zsb[:, mi, :], ps3[:],
                            dz2_sb[:, nt * 4 + mi:nt * 4 + mi + 1])
                    dst = z2bA[nt] if nt < NT_A else z2bB[nt - NT_A]
                    nc.sync.dma_start(dst, zsb[:])

                    if nt == NT_A - 1:
                        nc.gpsimd.collective_compute(
                            "AllGather", mybir.AluOpType.bypass,
                            ins=[z2bA[:]], outs=[z2gA[:]],
                            replica_groups=[list(range(NCORES))])
                    elif nt == NT - 1:
                        nc.gpsimd.collective_compute(
                            "AllGather", mybir.AluOpType.bypass,
                            ins=[z2bB[:]], outs=[z2gB[:]],
                            replica_groups=[list(range(NCORES))])
            z1_cm.__exit__(None, None, None)

            # ---- Phase 5: outT = dinv*contract(z2', C) + b2 (x) s --------
            # Hand-rolled k-outer loop: one PSUM bank per n-tile, so the
            # first gathered z2 tile starts compute while later gathers
            # are still in flight.
            with ExitStack() as ctx:
                a2_kxm = ctx.enter_context(
                    tc.tile_pool(name="a2_kxm", bufs=8))
                a2_kxn = ctx.enter_context(
                    tc.tile_pool(name="a2_kxn", bufs=32))
                a2_red = ctx.enter_context(tc.tile_pool(name="a2_red",
                                                        bufs=8))
                a2_ps = ctx.enter_context(
                    tc.tile_pool(name="a2_ps", bufs=1, space="PSUM"))

                psums = [a2_ps.tile([P, 512], f32, name=f"a2ps{n}")
                         for n in range(NT)]
                for q in range(KT):
                    t, g = divmod(q, NCORES)
                    src = z2gA[g, t] if t < NT_A else z2gB[g, t - NT_A]
                    zt = a2_kxm.tile([P, 4, COUT], f16, tag="z2kxm")
                    nc.sync.dma_start(zt[:], src)
                    ats = []
                    for n in range(NT):
                        at = a2_kxn.tile([P, 4, 512], f8, tag="a2A")
                        nc.sync.dma_start(at[:], Ab[q, n])
                        ats.append(at)
                    for ks in range(4):
                        for n in range(NT):
                            nc.tensor.matmul(
                                psums[n][:], zt[:, ks], ats[n][:, ks],
                                start=(q == 0 and ks == 0),
                                stop=(q == KT - 1 and ks == 3))

                for n in range(NT):
                    n0 = n * 512
                    tmp = a2_red.tile([P, 512], f32, tag="a2t")
                    osb = a2_red.tile([P, 512], f32, tag="a2o")
                    nc.vector.tensor_mul(tmp[:], psums[n][:],
                                         d_sb[:, n0:n0 + 512])
                    nc.vector.scalar_tensor_tensor(
                        osb[:], s_sb[:, n0:n0 + 512],
                        b2_sb[:, 0:1], tmp[:],
                        op0=Alu.mult, op1=Alu.add)
                    nc.sync.dma_start(outT[:, 0, n0:n0 + 512], osb[:])

    nc.compile()
    return nc


def _preprocess(x, edge_index, W1, b1, W2, b2):
    import ml_dtypes

    x = np.asarray(x, dtype=np.float32)
    edge_index = np.asarray(edge_index)
    W1 = np.asarray(W1, dtype=np.float32)
    b1 = np.asarray(b1, dtype=np.float32)
    W2 = np.asarray(W2, dtype=np.float32)
    b2 = np.asarray(b2, dtype=np.float32)

    row = edge_index[0].astype(np.int64)
    col = edge_index[1].astype(np.int64)

    deg = np.bincount(col, minlength=N_REAL).astype(np.float32) + 1.0
    dinv = 1.0 / np.sqrt(deg)

    idx = np.arange(N_REAL, dtype=np.int64)
    pad_id = (idx // RBLK) * BLK + idx % RBLK  # real -> padded node id

    # Dense count matrix, transposed: CT[src, dst] = A[dst, src] + I
    CT = np.zeros((NPAD, NPAD), dtype=np.uint8)
    np.add.at(CT, (pad_id[row], pad_id[col]), 1)
    CT[pad_id, pad_id] += 1
    assert CT.max() <= 16, "count exceeds exact fp8e4m3 integer range"

    # s[c] = sum_r A_hat[c, r]; dinv at padded positions -> 0
    s_real = dinv * (np.bincount(col, weights=dinv[row],
                                 minlength=N_REAL).astype(np.float32) + dinv)
    s_pad = np.zeros(NPAD, dtype=np.float32)
    s_pad[pad_id] = s_real
    dinv_pad = np.zeros(NPAD, dtype=np.float32)
    dinv_pad[pad_id] = dinv

    x_pad = np.zeros((NPAD, CIN), dtype=np.float16)
    x_pad[pad_id] = x.astype(np.float16)
    # xT tile layout: [mt][p][kt][ml] = x_pad[mt*512 + ml, kt*128 + p]
    xT_t = np.ascontiguousarray(
        x_pad.reshape(KT, 512, CIN // P, P).transpose(0, 3, 2, 1))

    W1_t = np.ascontiguousarray(
        W1.astype(np.float16).reshape(CIN // P, P, CHID).transpose(1, 0, 2))
    W2_t = np.ascontiguousarray(
        W2.astype(np.float16).reshape(CHID // P, P, COUT).transpose(1, 0, 2))
    b1_t = np.ascontiguousarray(b1.reshape(CHID // P, P).T)
    b2_t = np.ascontiguousarray(b2.reshape(COUT // P, P).T)
    # node-major per-partition dinv: [p][j] = dinv_pad[j*128 + p]
    dz1_t = np.ascontiguousarray(dinv_pad.reshape(NPAD // P, P).T)

    in_maps = []
    for g in range(NCORES):
        C_g = CT[:, g * BLK:(g + 1) * BLK]
        # [kt][nt][p][s][n] = C_g[kt*512 + s*128 + p, nt*512 + n],
        # then permute the kt axis into the device's q-order
        # (q -> physical kt = (q % NCORES) * NT + q // NCORES).
        perm = [(q % NCORES) * NT + q // NCORES for q in range(KT)]
        A_t = np.ascontiguousarray(
            C_g.reshape(KT, 4, P, NT, 512).transpose(0, 3, 2, 1, 4)[perm]
        ).astype(ml_dtypes.float8_e4m3)
        s_loc = s_pad[g * BLK:(g + 1) * BLK]
        d_loc = dinv_pad[g * BLK:(g + 1) * BLK]
        s_b = np.ascontiguousarray(
            np.broadcast_to(s_loc, (P, BLK))).astype(np.float32)
        d_b = np.ascontiguousarray(
            np.broadcast_to(d_loc, (P, BLK))).astype(np.float32)
        dz2_t = np.ascontiguousarray(d_loc.reshape(BLK // P, P).T)
        in_maps.append(dict(xT=xT_t, W1=W1_t, W2=W2_t, Ab=A_t,
                            sbc=s_b, dbc=d_b, dz1=dz1_t, dz2=dz2_t,
                            b1c=b1_t, b2c=b2_t))
    return in_maps


def _run(inputs, trace=False):
    global _compiled
    if _compiled is None:
        _compiled = _build_nc()
    nc = _compiled
    from concourse.bass_utils import run_bass_kernel_spmd

    in_maps = _preprocess(**inputs)
    res = run_bass_kernel_spmd(nc, in_maps, list(range(NCORES)), trace=trace)
    out = np.empty((N_REAL, COUT), dtype=np.float32)
    for g in range(NCORES):
        out[g * RBLK:(g + 1) * RBLK] = res.results[g]["outT"][:, 0, :RBLK].T
    return out, res


def kernel(**inputs) -> np.ndarray:
    out, _ = _run(inputs, trace=False)
    return out
